# revision 5
# baseline (speedup 1.0000x reference)
"""Symmetric brute-force KNN density estimator on 8 Trainium2 NeuronCores.

reference math:
    dist[i, j] = ||x_i - x_j||_2 over features [8192, 1024]
    kth[i] = 6th smallest of dist[i, :]  (self-distance included)
    out[i] = 1 / (kth[i] + 1e-8)

v3 strategy — full symmetry (circulant blocks + quadrant splits + host merge):
    Rank rows by T[i,j] = 2G[i,j] - (sq[j] - sbar); d2 = (sq[i]+sbar) - T.
    8 row-blocks of 1024. Core c computes, via fp8 DoubleRow matmuls:
      * blocks c+1..c+3 fully (24 [128,1024] PSUM groups), each mirrored;
      * the c+4 block's diagonal quadrants (8 [128,512] half-groups, all
        mirrored) — the paired core covers the anti-diagonal quadrants (its
        ft buffer has the halves swapped on the host);
      * the diagonal block's UL/LR quadrants fully plus UR mirrored
        (12 half-groups).
    Mirroring: the scalar engine copies raw-2G PSUM to SBUF bf16 adding the
    per-partition bias -(sq_i - sbar) (exactly the transposed tile's column
    bias), then dma_start_transpose scatters [128,128] pieces into mirror
    storage with a contiguous destination per source tile (a strided dest
    produces wrong output on HW). DVE max8 scans computed PSUM groups
    (FD=1024) and mirror banks (strided SBUF bf16 reads). The host merges
    80 top-8 candidate sets per core (10 sets per row, asserted) and does
    the sqrt/reciprocal recovery with exact fp32 norms.
"""

import os

import numpy as np
import ml_dtypes

N = 8192
D = 1024
NCORES = 8
NB = 8            # row/col blocks of 1024
BLK = N // NB     # 1024
KC = D // 128     # 8 contraction chunks -> 4 DoubleRow pairs
KP = KC // 2      # 4
RT = BLK // 128   # 8 row chunks per block
K_ORD = 5         # 6th largest/smallest
EPS = 1e-8
WARMUP_MM = 18
NSLOT = 5         # ft column-block slots: c+1, c+2, c+3, c+4, c(diag)
NMIR = 3          # full-block mirrored slots

# canonical device-order work lists (identical for every core; SPMD)
#   ('full', bi, r)          : [128,1024] group, slot bi in 0..2, mirrored
#   ('k4',   r)              : [128,512] group, slot 3, col off (r//4)*512, mirrored
#   ('dq',   r, off, mir)    : diag slot 4 half-group
GROUPS = (
    [("k4", r) for r in range(RT)]
    + [("dfull", r) for r in range(4)]
    + [("full", bi, r) for bi in range(NMIR) for r in range(RT)]
    + [("dq", r, 512, False) for r in range(4, 8)]
)
# mirror banks (scanned after their source transposes land). Full-block
# banks are split into r 0..3 / 4..7 halves so the first half drains while
# the block's later groups still run.
MBANKS = (
    [("mk", h, q) for h in range(2) for q in range(4)]
    + [("md", q) for q in range(4)]
    + [("mfa", bi, q) for bi in range(2) for q in range(RT)]
    + [("mfb", bi, q) for bi in range(2) for q in range(RT)]
    + [("mq", j2, q) for j2 in range(4) for q in range(RT)]
)
N_COMP = len(GROUPS)            # 44
NSETS = N_COMP + len(MBANKS)    # 44 + 36 = 80

TRACE = bool(int(os.environ.get("KNN_TRACE", "0")))
LAST_EXEC_NS = None


def _build_nc():
    import concourse.mybir as mybir
    from concourse import bacc
    from concourse.tile import TileContext

    dt = mybir.dt
    nc = bacc.Bacc(None, target_bir_lowering=False, enable_partition_id=False)

    qt_d = nc.dram_tensor("qt", [128, KP * 2 * BLK], dt.float8e4, kind="ExternalInput")
    ft_d = nc.dram_tensor("ft", [NSLOT, 128, KP * 2 * BLK], dt.float8e4, kind="ExternalInput")
    sqc_d = nc.dram_tensor("sqc", [1, NSLOT * BLK], dt.bfloat16, kind="ExternalInput")
    sqa_d = nc.dram_tensor("sqa", [128, RT], dt.float32, kind="ExternalInput")
    cand_d = nc.dram_tensor("cand", [128, NSETS * 8], dt.float32, kind="ExternalOutput")

    DR = mybir.MatmulPerfMode.DoubleRow

    with TileContext(nc) as tc:
        with (
            tc.tile_pool(name="persist", bufs=1) as persist,
            tc.tile_pool(name="cp", bufs=3) as cpp,
            tc.tile_pool(name="cph", bufs=4) as cphp,
            tc.tile_pool(name="psum", bufs=4, space="PSUM") as psum,
        ):
            qt_s = persist.tile([128, KP, 2, BLK], dt.float8e4)
            ft_s = persist.tile([128, NSLOT, KP, 2, BLK], dt.float8e4)
            sqc_s = persist.tile([1, NSLOT * BLK], dt.bfloat16)
            sqa_s = persist.tile([128, RT], dt.float32)
            ones_s = persist.tile([1, 128], dt.bfloat16)
            warm_s = persist.tile([128, 512], dt.bfloat16)
            # contiguous-per-source-tile mirror storage (see module docstring)
            mir_f = persist.tile([128, NMIR, RT, RT, 128], dt.bfloat16)
            mir_k = persist.tile([128, RT, 4, 128], dt.bfloat16)
            mir_d = persist.tile([128, 4, 4, 128], dt.bfloat16)
            cand = persist.tile([128, NSETS * 8], dt.float32)

            # PE warm-up during the initial DMA window (HAM clock gate)
            nc.vector.memset(ones_s, 1.0)
            nc.vector.memset(warm_s, 0.0)
            wtile = psum.tile([128, 1024], dt.float32, tag="ps")
            for i in range(WARMUP_MM):
                nc.tensor.matmul(wtile[:, 0:512], lhsT=warm_s[:, 0:128], rhs=warm_s,
                                 start=(i == 0), stop=(i == WARMUP_MM - 1))

            qt_r = qt_d[:, :].rearrange("p (k t j) -> p k t j", k=KP, t=2)
            ft_r = [ft_d[b].rearrange("p (k t j) -> p k t j", k=KP, t=2)
                    for b in range(NSLOT)]
            nc.sync.dma_start(sqc_s, sqc_d[:, :])
            nc.sync.dma_start(sqa_s, sqa_d[:, :])
            nc.sync.dma_start(qt_s[:, 0:2], qt_r[:, 0:2])
            nc.gpsimd.dma_start(ft_s[:, 3, 0:2], ft_r[3][:, 0:2])
            nc.sync.dma_start(qt_s[:, 2:4], qt_r[:, 2:4])
            nc.sync.dma_start(ft_s[:, 3, 2:4], ft_r[3][:, 2:4])
            nc.sync.dma_start(ft_s[:, 4], ft_r[4])
            nc.sync.dma_start(ft_s[:, 0], ft_r[0])
            nc.gpsimd.dma_start(ft_s[:, 1], ft_r[1])
            nc.gpsimd.dma_start(ft_s[:, 2], ft_r[2])

            # device emission must match the canonical order: computed sets
            # first (GROUPS order), then mirror banks (MBANKS order). Mirror
            # scans are interleaved for pipelining but their cand slots are
            # pre-assigned from the MBANKS order.
            comp_slots = {}
            for i, g in enumerate(GROUPS):
                comp_slots[g] = i
            bank_slots = {}
            for i, mb in enumerate(MBANKS):
                bank_slots[mb] = N_COMP + i

            def scan_bank_at(mb):
                s = bank_slots[mb]
                out = cand[:, s * 8:(s + 1) * 8]
                kind = mb[0]
                if kind == "mfa":
                    _, bi, q = mb
                    nc.vector.max(out=out, in_=mir_f[:, bi, 0:4, q, :])
                elif kind == "mfb":
                    _, bi, q = mb
                    nc.vector.max(out=out, in_=mir_f[:, bi, 4:8, q, :])
                elif kind == "mq":
                    _, j2, q = mb
                    nc.vector.max(out=out, in_=mir_f[:, 2, 2 * j2:2 * j2 + 2, q, :])
                elif kind == "mk":
                    _, h, q = mb
                    nc.vector.max(out=out, in_=mir_k[:, 4 * h:4 * h + 4, q, :])
                else:
                    _, q = mb
                    nc.vector.max(out=out, in_=mir_d[:, 0:4, q, :])

            tcount = [0]

            def do_group(g):
                kind = g[0]
                if kind == "full":
                    _, bi, r = g
                    slot, off, w, mir = bi, 0, 1024, True
                elif kind == "k4":
                    _, r = g
                    slot, off, w, mir = 3, (r // 4) * 512, 512, True
                elif kind == "dfull":
                    _, r = g
                    slot, off, w, mir = 4, 0, 1024, True
                else:
                    _, r, off, mir = g
                    slot, w = 4, 512
                ps = psum.tile([128, 1024], dt.float32, tag="ps")
                nh = w // 512
                for kp in range(KP):
                    for half in range(nh):
                        nc.tensor.matmul(
                            ps[:, half * 512:(half + 1) * 512],
                            lhsT=qt_s[:, kp, :, r * 128:(r + 1) * 128],
                            rhs=ft_s[:, slot, kp, :, off + half * 512: off + (half + 1) * 512],
                            start=(kp == 0), stop=False, perf_mode=DR)
                # fold the computed tile's column bias immediately (PE never
                # waits); the mirror copy below then carries an extra
                # -(sq_j-sbar) which is per-partition in the mirror bank, so
                # it does not affect ranking and the host adds it back.
                for half in range(nh):
                    c0 = slot * BLK + off + half * 512
                    nc.tensor.matmul(
                        ps[:, half * 512:(half + 1) * 512],
                        lhsT=ones_s, rhs=sqc_s[:, c0:c0 + 512],
                        start=False, stop=(half == nh - 1))
                if mir:
                    if kind == "full":
                        cp = cpp.tile([128, 1024], dt.bfloat16, tag="cp")
                        nc.scalar.add(cp, ps, sqa_s[:, r:r + 1])
                        dst = mir_f[:, slot, r, :, :]
                    elif kind == "dfull":
                        # only the UR half (cols 512:1024) is mirrored -> LL
                        cp = cphp.tile([128, 512], dt.bfloat16, tag="cph")
                        nc.scalar.add(cp, ps[:, 512:1024], sqa_s[:, r:r + 1])
                        dst = mir_d[:, r, :, :]
                    else:
                        cp = cphp.tile([128, 512], dt.bfloat16, tag="cph")
                        nc.scalar.add(cp, ps[:, 0:512], sqa_s[:, r:r + 1])
                        dst = mir_k[:, r, :, :] if kind == "k4" else mir_d[:, r, :, :]
                    if kind == "full":
                        eng = nc.sync if (tcount[0] % 2 == 0) else nc.scalar
                        tcount[0] += 1
                    else:
                        eng = nc.sync
                    eng.dma_start_transpose(dst, cp)
                s = comp_slots[g]
                if kind in ("full", "k4"):
                    # scan the bf16 copy: cheaper DVE read than PSUM and the
                    # PSUM bank frees right after the scalar-engine copy; the
                    # copy's extra -(sq_i-sbar) is per-partition (per row), so
                    # ranking is unchanged and the host adds it back
                    nc.vector.max(out=cand[:, s * 8:(s + 1) * 8], in_=cp)
                else:
                    nc.vector.max(out=cand[:, s * 8:(s + 1) * 8], in_=ps[:, 0:w])

            # schedule: emit groups; after each group, drain one pending bank
            ready_after = {}
            for bi in range(2):
                ready_after[("full", bi, 3)] = [("mfa", bi, q) for q in range(RT)]
                ready_after[("full", bi, 7)] = [("mfb", bi, q) for q in range(RT)]
            for j2 in range(4):
                ready_after[("full", 2, 2 * j2 + 1)] = [("mq", j2, q) for q in range(RT)]
            ready_after[("k4", 3)] = [("mk", 0, q) for q in range(4)]
            ready_after[("k4", 7)] = [("mk", 1, q) for q in range(4)]
            ready_after[("dfull", 3)] = [("md", q) for q in range(4)]

            queue = []
            delay = []         # one-group delay before banks become poppable
            for g in GROUPS:
                do_group(g)
                queue.extend(delay)
                delay = ready_after.get(g, [])
                budget = 1250
                while queue and budget > 0:
                    budget -= 400 if queue[0][0] == "mq" else 660
                    scan_bank_at(queue.pop(0))
            queue.extend(delay)
            for mb in queue:
                scan_bank_at(mb)

            nc.sync.dma_start(cand_d[:, :], cand)

    nc.finalize()
    return nc


def kernel(features):
    global LAST_EXEC_NS
    from concourse.bass_utils import run_bass_kernel_spmd

    f32 = np.ascontiguousarray(np.asarray(features, dtype=np.float32))
    assert f32.shape == (N, D)

    sq = np.einsum("nd,nd->n", f32, f32, dtype=np.float32)
    sbar = float(sq.mean())

    ftq = f32.T.astype(ml_dtypes.float8_e4m3fn)                   # [D, N] fp8
    ft2 = (ftq.astype(np.float32) * 2.0).astype(ml_dtypes.float8_e4m3fn)
    sqd = (-(sq - sbar)).astype(np.float32)

    def chunk_cols(src, cols):
        blk = src[:, cols]                                        # [D, BLK]
        return blk.reshape(KP, 2, 128, BLK).transpose(2, 0, 1, 3).reshape(128, KP * 2 * BLK)

    in_maps = []
    col_tables = []
    for c in range(NCORES):
        blocks = [(c + o) % NB for o in [1, 2, 3, 4, 0]]
        # slot 3 (k4): swap column halves for c >= 4 so the shared device
        # program computes complementary quadrants on the two paired cores
        slot_cols = []
        for si, b in enumerate(blocks):
            cols = np.arange(b * BLK, (b + 1) * BLK)
            if si == 3 and c >= 4:
                cols = np.concatenate([cols[512:], cols[:512]])
            slot_cols.append(cols)
        col_tables.append(slot_cols)
        qt = np.ascontiguousarray(chunk_cols(ftq, np.arange(c * BLK, (c + 1) * BLK)))
        ft = np.ascontiguousarray(
            np.stack([chunk_cols(ft2, cols) for cols in slot_cols], axis=0))
        sqc = np.ascontiguousarray(
            np.concatenate([sqd[cols] for cols in slot_cols])[None, :].astype(ml_dtypes.bfloat16))
        sqa = np.ascontiguousarray(
            -(sq[c * BLK:(c + 1) * BLK] - sbar).reshape(RT, 128).T.astype(np.float32))
        in_maps.append({"qt": qt, "ft": ft, "sqc": sqc, "sqa": sqa})

    nc = _build_nc()
    res = run_bass_kernel_spmd(nc, in_maps, core_ids=list(range(NCORES)), trace=TRACE)
    LAST_EXEC_NS = res.exec_time_ns

    # host merge: per global 128-row chunk, gather its candidate sets
    from collections import defaultdict
    chunk_sets = defaultdict(list)
    for c in range(NCORES):
        arr = np.asarray(res.results[c]["cand"]).reshape(128, NSETS, 8)
        slot_cols = col_tables[c]
        # computed sets: rows are always the core's own rows
        for i, g in enumerate(GROUPS):
            r = g[2] if g[0] == "full" else g[1]
            rows0 = c * BLK + r * 128
            vals = arr[:, i, :]
            if g[0] in ("full", "k4"):
                vals = vals + (sq[rows0:rows0 + 128] - sbar).astype(np.float32)[:, None]
            chunk_sets[rows0 // 128].append(vals)
        # mirror banks: rows = source columns of the transposed tiles
        for j, mb in enumerate(MBANKS):
            i = N_COMP + j
            if mb[0] in ("mfa", "mfb"):
                _, bi, q = mb
                col0 = slot_cols[bi][q * 128]
            elif mb[0] == "mq":
                _, j2, q = mb
                col0 = slot_cols[2][q * 128]
            elif mb[0] == "mk":
                _, h, q = mb
                col0 = slot_cols[3][h * 512 + q * 128]
            else:
                _, q = mb
                col0 = slot_cols[4][512 + q * 128]
            assert col0 % 128 == 0
            adj = (sq[col0:col0 + 128] - sbar).astype(np.float32)
            chunk_sets[col0 // 128].append(arr[:, i, :] + adj[:, None])

    t6 = np.empty(N, dtype=np.float32)
    for ch in range(N // 128):
        sets = chunk_sets[ch]
        assert len(sets) in (14, 15), (ch, len(sets))
        vals = np.concatenate(sets, axis=1)            # [128, 80]
        t6[ch * 128:(ch + 1) * 128] = np.partition(
            vals, vals.shape[1] - 1 - K_ORD, axis=1)[:, vals.shape[1] - 1 - K_ORD]
    kd = np.maximum((sq + sbar) - t6, 0.0)
    dens = 1.0 / (np.sqrt(kd) + EPS)
    return dens.astype(np.float32)[:, None]
